# revision 8
# baseline (speedup 1.0000x reference)
"""Trainium2 Bass kernel for nn_MultiHeadAttention_76321568850683.

Multi-head attention (b=4, t=32, n=288 tokens, d=128, H=4 heads, hd=32) with
block-wise differential dropout on the softmax weights:

    q,k,v = x@Wq, x@Wk, x@Wv   (per (b,t) slice)
    S_h   = q_h k_h^T / sqrt(32)
    w     = softmax(S) * mask * scale ;  mask = drop_u < keep_p(q,k)
    out   = (w @ v) @ Wo + bo

Sharding: data-parallel over the 128 (b,t) pairs -> 16 per NeuronCore x 8.

Per-core dataflow ("transposed" layout, feature/key on partitions):
  - xT (d,n); qT,kT = Wq^T@xT, Wk^T@xT (PE, float32r full-rate fp32)
  - scores transposed: ST_h (k,q) chunks: k in [0,128),[128,256) via K=32
    row-tiled matmuls (4 heads share the qT moving stream); k in [256,288)
    via one block-diagonal matmul (KBD trick) giving a head-stacked tile
  - E = exp(ST/sqrt(hd)) on ACT (fp16); maskbits = (drop_uT < p) on GPSIMD
    (fp16); EM = E*maskbits on DVE (fp16 2x mode)
  - denominators: R = ones^T @ E per head band (fp16 col-tiled matmuls,
    fp32 PSUM accumulation); ctxT_h = v_h^T @ EM likewise
  - ctxn = ctxT / R (DVE); outT = Wo^T @ ctxn + bo (f32r + ACT bias)
  - host re-transposes outT to (b,t,n,d)
float32r (tf32 on the PE) cannot col-tile, hence fp16 for R/ctx; all
rounding sites sit after the softmax where errors average out.
"""
import sys

sys.path.insert(0, "/opt/trn_rl_repo")

from contextlib import ExitStack

import numpy as np

import concourse.bass as bass
import concourse.bacc as bacc
import concourse.tile as tile
from concourse import mybir
from concourse.bass_utils import run_bass_kernel_spmd

F32 = mybir.dt.float32
F32R = mybir.dt.float32r
F16 = mybir.dt.float16
EXP = mybir.ActivationFunctionType.Exp
IDENT = mybir.ActivationFunctionType.Identity
MULT = mybir.AluOpType.mult
ISLT = mybir.AluOpType.is_lt

N_CORES = 8
B, T, N, D = 4, 32, 288, 128
H, HD = 4, 32
BT = B * T
BT_PER_CORE = BT // N_CORES
KEEP_X = 0.9
KEEP_LQ = 0.7
INV_SQRT_HD = float(1.0 / np.sqrt(HD))


def build_nc(bt_per_core: int, nxt: int) -> bass.Bass:
    """Per-core program; nxt = num_x_tokens (mask region boundary)."""
    nc = bacc.Bacc("TRN2", target_bir_lowering=False, debug=False)

    xT_d = nc.dram_tensor("xT", [bt_per_core, D, N], F32R, kind="ExternalInput").ap()
    du01_d = nc.dram_tensor(
        "du01", [bt_per_core, 2, H, 128, N], F32, kind="ExternalInput"
    ).ap()
    du2_d = nc.dram_tensor("du2", [bt_per_core, 128, N], F32, kind="ExternalInput").ap()
    wq_d = nc.dram_tensor("Wq", [D, D], F32R, kind="ExternalInput").ap()
    wk_d = nc.dram_tensor("Wk", [D, D], F32R, kind="ExternalInput").ap()
    wv_d = nc.dram_tensor("Wv", [D, D], F32R, kind="ExternalInput").ap()
    wo_d = nc.dram_tensor("Wo", [D, D], F32R, kind="ExternalInput").ap()
    bo_d = nc.dram_tensor("bo", [D, 1], F32, kind="ExternalInput").ap()
    sel_d = nc.dram_tensor("sel", [32, 128], F16, kind="ExternalInput").ap()
    obd_d = nc.dram_tensor("onesbd", [D, D], F16, kind="ExternalInput").ap()
    kbdz_d = nc.dram_tensor("kbdz", [D, D], F32R, kind="ExternalInput").ap()
    out_d = nc.dram_tensor("outT", [bt_per_core, D, N], F32, kind="ExternalOutput").ap()

    with tile.TileContext(nc) as tc, ExitStack() as ctx:
        consts = ctx.enter_context(tc.tile_pool(name="consts", bufs=1))
        xp = ctx.enter_context(tc.tile_pool(name="xp", bufs=3))
        qkp = ctx.enter_context(tc.tile_pool(name="qkp", bufs=2))
        vp = ctx.enter_context(tc.tile_pool(name="vp", bufs=2))
        dup = ctx.enter_context(tc.tile_pool(name="dup", bufs=6))
        ep = ctx.enter_context(tc.tile_pool(name="ep", bufs=5))
        emp = ctx.enter_context(tc.tile_pool(name="emp", bufs=5))
        mbp = ctx.enter_context(tc.tile_pool(name="mbp", bufs=3))
        outp = ctx.enter_context(tc.tile_pool(name="outp", bufs=2))
        ps_big = ctx.enter_context(tc.tile_pool(name="ps_big", bufs=2, space="PSUM"))
        ps_sm = ctx.enter_context(tc.tile_pool(name="ps_sm", bufs=1, space="PSUM"))

        wq_s = consts.tile([D, D], F32R)
        nc.sync.dma_start(wq_s[:], wq_d[:])
        wk_s = consts.tile([D, D], F32R)
        nc.sync.dma_start(wk_s[:], wk_d[:])
        wv_s = consts.tile([D, D], F32R)
        nc.sync.dma_start(wv_s[:], wv_d[:])
        wo_s = consts.tile([D, D], F32R)
        nc.sync.dma_start(wo_s[:], wo_d[:])
        bo_s = consts.tile([D, 1], F32)
        nc.sync.dma_start(bo_s[:], bo_d[:])
        sel_s = consts.tile([32, 128], F16)
        nc.sync.dma_start(sel_s[:], sel_d[:])
        obd_s = consts.tile([D, D], F16)
        nc.sync.dma_start(obd_s[:], obd_d[:])
        kbd_s = consts.tile([D, D], F32R)  # block-diag kT chunk2; off-diag stays 0
        nc.sync.dma_start(kbd_s[:], kbdz_d[:])
        ones_s = consts.tile([128, 32], F16)
        nc.vector.memset(ones_s[:], 1.0)

        for bt in range(bt_per_core):
            x_t = xp.tile([D, N], F32R)
            nc.sync.dma_start(x_t[:], xT_d[bt])

            # ---- q/k projections: qT,kT (d,n) = W^T @ xT ----
            ps_qk = ps_big.tile([128, 1024], F32, name=f"ps_qk_{bt}", tag="s")
            nc.tensor.matmul(ps_qk[:, 0:N], wq_s[:], x_t[:], start=True, stop=True)
            nc.tensor.matmul(
                ps_qk[:, 512 : 512 + N], wk_s[:], x_t[:], start=True, stop=True
            )
            qk_t = qkp.tile([128, 2 * N], F32R)
            nc.scalar.copy(
                qk_t.rearrange("p (b q) -> p b q", b=2),
                ps_qk.rearrange("p (b q) -> p b q", b=2)[:, :, 0:N],
            )
            qT = qk_t[:, 0:N]
            kT = qk_t[:, N : 2 * N]

            # ---- v projection, (k,d) layout; chunk2 plain then replicated ----
            ps_v = ps_sm.tile([128, 512], F32)
            nc.tensor.matmul(
                ps_v[0:128, 0:128], x_t[:, 0:128], wv_s[:], start=True, stop=True
            )
            nc.tensor.matmul(
                ps_v[0:128, 128:256], x_t[:, 128:256], wv_s[:], start=True, stop=True
            )
            nc.tensor.matmul(
                ps_v[0:32, 256:384], x_t[:, 256:288], wv_s[:], start=True, stop=True
            )
            v_t = vp.tile([128, 256], F16, name=f"v_{bt}", tag="v")
            nc.vector.tensor_copy(v_t[:], ps_v[:, 0:256])
            v2_t = vp.tile([32, 128], F16, name=f"v2_{bt}", tag="v2")
            nc.vector.tensor_copy(v2_t[:], ps_v[0:32, 256:384])
            # replicate v chunk2 to all 4 partition bands: v2rep = sel^T @ v2
            nc.tensor.matmul(
                ps_v[:, 384:512], sel_s[:], v2_t[:], start=True, stop=True
            )
            v2r_t = vp.tile([128, 128], F16, name=f"v2r_{bt}", tag="v2r")
            nc.vector.tensor_copy(v2r_t[:], ps_v[:, 384:512])
            vbd_t = vp.tile([128, 128], F16, name=f"vbd_{bt}", tag="vbd")
            nc.vector.tensor_tensor(vbd_t[:], v2r_t[:], obd_s[:], MULT)

            # ---- block-diag kT chunk2 for the stacked c2 score matmul ----
            for h in range(H):
                b0 = 32 * h
                nc.vector.tensor_copy(
                    kbd_s[b0 : b0 + 32, b0 : b0 + 32], kT[b0 : b0 + 32, 256:288]
                )

            # ---- scores + exp + mask + EM per chunk ----
            e_list = []
            em_list = []
            for c in range(2):
                ko = 128 * c
                ps_s0 = ps_big.tile([128, 1024], F32, name=f"ps_s0_{bt}_{c}", tag="s")
                ps_s1 = ps_big.tile([128, 1024], F32, name=f"ps_s1_{bt}_{c}", tag="s")
                for h in range(H):
                    pt = ps_s0 if h < 2 else ps_s1
                    off = 512 * (h % 2)
                    nc.tensor.matmul(
                        pt[:, off : off + N],
                        kT[32 * h : 32 * h + 32, ko : ko + 128],
                        qT[32 * h : 32 * h + 32, :],
                        start=True,
                        stop=True,
                        tile_position=(32 * h, 0),
                    )
                e_t = ep.tile([128, H * N], F16, name=f"e_{bt}_{c}", tag="e")
                for p, ps_s in enumerate((ps_s0, ps_s1)):
                    nc.scalar.activation(
                        e_t[:, 2 * N * p : 2 * N * (p + 1)].rearrange(
                            "p (b q) -> p b q", b=2
                        ),
                        ps_s.rearrange("p (b q) -> p b q", b=2)[:, :, 0:N],
                        EXP,
                        bias=0.0,
                        scale=INV_SQRT_HD,
                    )
                du_t = dup.tile([128, H * N], F32, name=f"du_{bt}_{c}", tag="du")
                nc.sync.dma_start(
                    du_t.rearrange("p (h q) -> p h q", h=H),
                    du01_d[bt, c].rearrange("h k q -> k h q"),
                )
                mb_t = mbp.tile([128, H * N], F16, name=f"mb_{bt}_{c}", tag="mb")
                du_r = du_t.rearrange("p (h q) -> p h q", h=H)
                mb_r = mb_t.rearrange("p (h q) -> p h q", h=H)
                if nxt > 0:
                    nc.gpsimd.tensor_scalar(
                        mb_r[:, :, 0:nxt], du_r[:, :, 0:nxt], KEEP_X, None, ISLT
                    )
                if nxt < N:
                    nc.gpsimd.tensor_scalar(
                        mb_r[:, :, nxt:N], du_r[:, :, nxt:N], KEEP_LQ, None, ISLT
                    )
                em_t = emp.tile([128, H * N], F16, name=f"em_{bt}_{c}", tag="em")
                nc.vector.tensor_tensor(em_t[:], e_t[:], mb_t[:], MULT)
                e_list.append(e_t)
                em_list.append(em_t)

            # chunk 2 (k in [256,288)): one block-diag matmul, head-stacked out
            ps_s2 = ps_big.tile([128, 1024], F32, name=f"ps_s2_{bt}", tag="s")
            nc.tensor.matmul(ps_s2[:, 0:N], kbd_s[:], qT[:], start=True, stop=True)
            e2_t = ep.tile([128, N], F16, name=f"e2_{bt}", tag="e2")
            nc.scalar.activation(
                e2_t[:], ps_s2[:, 0:N], EXP, bias=0.0, scale=INV_SQRT_HD
            )
            du2_t = dup.tile([128, N], F32, name=f"du2_{bt}", tag="du2")
            nc.sync.dma_start(du2_t[:], du2_d[bt])
            mb2_t = mbp.tile([128, N], F16, name=f"mb2_{bt}", tag="mb2")
            nsx = max(0, min(nxt, N) - 256)  # x-rows within each head band
            if nsx == 0:
                nc.gpsimd.tensor_scalar(mb2_t[:], du2_t[:], KEEP_LQ, None, ISLT)
            else:
                for h in range(H):
                    b0 = 32 * h
                    if nxt > 0:
                        nc.gpsimd.tensor_scalar(
                            mb2_t[b0 : b0 + nsx, 0:nxt],
                            du2_t[b0 : b0 + nsx, 0:nxt],
                            KEEP_X,
                            None,
                            ISLT,
                        )
                    if nxt < N:
                        nc.gpsimd.tensor_scalar(
                            mb2_t[b0 : b0 + nsx, nxt:N],
                            du2_t[b0 : b0 + nsx, nxt:N],
                            KEEP_LQ,
                            None,
                            ISLT,
                        )
                    if nsx < 32:
                        nc.gpsimd.tensor_scalar(
                            mb2_t[b0 + nsx : b0 + 32, :],
                            du2_t[b0 + nsx : b0 + 32, :],
                            KEEP_LQ,
                            None,
                            ISLT,
                        )
            em2_t = emp.tile([128, N], F16, name=f"em2_{bt}", tag="em2")
            nc.vector.tensor_tensor(em2_t[:], e2_t[:], mb2_t[:], MULT)

            # ---- denominators R and context, band-major accumulation ----
            ps_r = ps_sm.tile([128, 512], F32)
            ps_c = ps_sm.tile([128, 512], F32)
            for h in range(H):
                b0 = 32 * h
                for c in range(2):
                    nc.tensor.matmul(
                        ps_r[b0 : b0 + 32, 0:N],
                        ones_s[0:128, :],
                        e_list[c][:, N * h : N * (h + 1)],
                        start=(c == 0),
                        stop=False,
                        skip_group_check=True,
                        tile_position=(0, b0),
                    )
                nc.tensor.matmul(
                    ps_r[b0 : b0 + 32, 0:N],
                    obd_s[:, b0 : b0 + 32],
                    e2_t[:],
                    start=False,
                    stop=True,
                    skip_group_check=True,
                    tile_position=(0, b0),
                )
                for c in range(2):
                    nc.tensor.matmul(
                        ps_c[b0 : b0 + 32, 0:N],
                        v_t[0:128, 128 * c + b0 : 128 * c + b0 + 32],
                        em_list[c][:, N * h : N * (h + 1)],
                        start=(c == 0),
                        stop=False,
                        skip_group_check=True,
                        tile_position=(0, b0),
                    )
                nc.tensor.matmul(
                    ps_c[b0 : b0 + 32, 0:N],
                    vbd_t[:, b0 : b0 + 32],
                    em2_t[:],
                    start=False,
                    stop=True,
                    skip_group_check=True,
                    tile_position=(0, b0),
                )

            # ---- normalize + output projection ----
            rr_t = outp.tile([128, N], F32, name=f"rr_{bt}", tag="rr")
            nc.vector.reciprocal(rr_t[:], ps_r[:, 0:N])
            ctxn_t = outp.tile([128, N], F32R, name=f"ctxn_{bt}", tag="ctxn")
            nc.vector.tensor_tensor(ctxn_t[:], ps_c[:, 0:N], rr_t[:], MULT)
            ps_o = ps_sm.tile([128, 512], F32)
            nc.tensor.matmul(ps_o[:, 0:N], wo_s[:], ctxn_t[:], start=True, stop=True)
            out_t = outp.tile([128, N], F32, name=f"out_{bt}", tag="out")
            nc.scalar.activation(out_t[:], ps_o[:, 0:N], IDENT, bias=bo_s[:], scale=1.0)
            nc.sync.dma_start(out_d[bt], out_t[:])

    nc.compile()
    return nc


def _prep_inputs(x, Wq, Wk, Wv, Wo, bo, drop_u, num_x_tokens):
    import ml_dtypes

    nxt = int(num_x_tokens)
    n2 = N * N
    nxx = nxt * nxt
    scale = n2 / (nxx * KEEP_X + (n2 - nxx) * KEEP_LQ)

    xT = np.ascontiguousarray(
        np.asarray(x, dtype=np.float32).reshape(BT, N, D).transpose(0, 2, 1)
    )
    duT = np.asarray(drop_u, dtype=np.float32).reshape(BT, H, N, N).transpose(0, 1, 3, 2)
    du01 = np.ascontiguousarray(
        duT[:, :, 0:256, :].reshape(BT, H, 2, 128, N).transpose(0, 2, 1, 3, 4)
    )
    du2 = np.ascontiguousarray(duT[:, :, 256:288, :].reshape(BT, 128, N))

    Wq = np.ascontiguousarray(np.asarray(Wq, dtype=np.float32))
    Wk = np.ascontiguousarray(np.asarray(Wk, dtype=np.float32))
    Wv = np.ascontiguousarray(np.asarray(Wv, dtype=np.float32))
    Wo_s = np.ascontiguousarray(np.asarray(Wo, dtype=np.float32) * np.float32(scale))
    bo_c = np.ascontiguousarray(np.asarray(bo, dtype=np.float32).reshape(D, 1))
    sel = np.zeros((32, 128), dtype=np.float16)
    for h in range(H):
        for i in range(32):
            sel[i, 32 * h + i] = 1.0
    onesbd = np.zeros((D, D), dtype=np.float16)
    for h in range(H):
        onesbd[32 * h : 32 * h + 32, 32 * h : 32 * h + 32] = 1.0
    kbdz = np.zeros((D, D), dtype=np.float32)

    in_maps = []
    for c in range(N_CORES):
        s = slice(c * BT_PER_CORE, (c + 1) * BT_PER_CORE)
        in_maps.append(
            {
                "xT": xT[s],
                "du01": du01[s],
                "du2": du2[s],
                "Wq": Wq,
                "Wk": Wk,
                "Wv": Wv,
                "Wo": Wo_s,
                "bo": bo_c,
                "sel": sel,
                "onesbd": onesbd,
                "kbdz": kbdz,
            }
        )
    return in_maps, nxt


def kernel(x, Wq, Wk, Wv, Wo, bo, drop_u, num_x_tokens, _trace=False, _nc_cache={}):
    in_maps, nxt = _prep_inputs(x, Wq, Wk, Wv, Wo, bo, drop_u, num_x_tokens)
    key = (BT_PER_CORE, nxt)
    if key not in _nc_cache:
        _nc_cache[key] = build_nc(BT_PER_CORE, nxt)
    nc = _nc_cache[key]
    res = run_bass_kernel_spmd(nc, in_maps, list(range(N_CORES)), trace=_trace)
    outT = np.concatenate([r["outT"] for r in res.results], axis=0)
    out = np.ascontiguousarray(
        outT.transpose(0, 2, 1).reshape(B, T, N, D), dtype=np.float32
    )
    if _trace:
        kernel._last_results = res
    return out
